# revision 25
# baseline (speedup 1.0000x reference)
"""Trainium2 Bass kernel for EntropySamplLoss.

Reference semantics (per image b):
  acts [N, P=320] viewed as [N, S=4, C=8, K=10] prototype groups
  ent[n, s, c] = normalized softmax entropy over the K protos of group (s, c)
  num[s, c]   = sum over pixels n with label c of ent[n, s, c]
  cnt[c]      = number of pixels with label c
  loss = mean over present (b, s, c) of num[s, c] / cnt[c]

Device kernel (data-parallel, one image per NeuronCore):
  per chunk of 1024 pixels (tile [128 part, 2560], 8 px per partition):
    E   = exp(x)                       (ACT)
    Z   = group-sum_k E                (DVE grouped tensor_reduce)
    xE  = x * E                        (GPSIMD)
    U   = group-sum_k xE               (DVE grouped tensor_reduce)
    logZ = ln(Z); rZ = exp(-logZ)      (ACT)
    ent_raw = logZ - U*rZ              (DVE)  [= ln(K) * normalized entropy]
    mask[px, (j,c)] = labels==c+1      (DVE is_equal vs broadcast iota)
    stats[ (j,c), (j',sc|ones) ] += mask^T @ [ent|1]   (PE matmul, PSUM accum)
  host: extract diagonal j==j', divide by ln(K), per-class means, final mean.
"""

import sys

if "/opt/trn_rl_repo" not in sys.path:
    sys.path.insert(0, "/opt/trn_rl_repo")

from contextlib import ExitStack

import numpy as np

import concourse.bacc as bacc
import concourse.bass as bass
import concourse.tile as tile
from concourse import mybir
from concourse.bass_utils import run_bass_kernel_spmd
from concourse.tile import add_dep_helper

# Problem shape (hardcoded per spec)
B, N, PP = 8, 65536, 320
S, C, K = 4, 8, 10
NCORES = 8

PX_PER_PART = 8          # pixels per partition ("j" slots)
PART = 128
PX_PER_CHUNK = PART * PX_PER_PART      # 1024
NCHUNK = N // PX_PER_CHUNK             # 64
FREE = PX_PER_PART * PP                # 2560
G = S * C                              # 32 groups per pixel
GF = PX_PER_PART * G                   # 256 group slots per partition
EW = G + 1                             # 33: ent cols + ones col
PH = 4                                 # chunks per ACT table-set phase group
MSHIFT = 12.0                          # shift for the silu(x-m) ~ (x-m)e^(x-m) trick
USE_GP_PAIRSUM = False

_CACHE = {}


def _patch_act_tables():
    """Make the combined exp+ln table set the only candidate for Exp/Ln so
    the table-load placement pass doesn't thrash between per-function sets
    (one ACT_TABLE_LOAD total instead of 2 per chunk)."""
    import concourse.hw_specs as hw_specs

    tabs = hw_specs.get_activation_tables("gen3")
    E = mybir.ActivationFunctionType.Exp
    L = mybir.ActivationFunctionType.Ln
    for name, funcs in tabs.items():
        if name != "natural_log_exp_and_others":
            funcs.discard(E)
            funcs.discard(L)


def _build():
    if "nc" in _CACHE:
        return _CACHE["nc"]

    _patch_act_tables()
    f32 = mybir.dt.float32
    nc = bacc.Bacc("TRN2", target_bir_lowering=False, debug=False, num_devices=NCORES)

    acts = nc.dram_tensor("acts", [NCHUNK, PART, FREE], f32, kind="ExternalInput").ap()
    labels = nc.dram_tensor(
        "labels", [NCHUNK, PART, PX_PER_PART], f32, kind="ExternalInput"
    ).ap()
    consts = nc.dram_tensor("consts", [C + 1], f32, kind="ExternalInput")
    stats_out = nc.dram_tensor(
        "stats", [PX_PER_PART * C, PX_PER_PART * EW], f32, kind="ExternalOutput"
    ).ap()
    stats2_out = nc.dram_tensor(
        "stats2", [PX_PER_PART * C, PX_PER_PART * G], f32, kind="ExternalOutput"
    ).ap()

    with tile.TileContext(nc) as tc:
        with ExitStack() as ctx:
            singles = ctx.enter_context(tc.tile_pool(name="singles", bufs=1))
            big = ctx.enter_context(tc.tile_pool(name="big", bufs=PH + 2))
            ebuf = ctx.enter_context(tc.tile_pool(name="ebuf", bufs=3))
            sybuf = ctx.enter_context(tc.tile_pool(name="sybuf", bufs=3))
            sy5buf = ctx.enter_context(tc.tile_pool(name="sy5buf", bufs=3))
            small = ctx.enter_context(tc.tile_pool(name="small", bufs=3))
            psum = ctx.enter_context(tc.tile_pool(name="psum", bufs=1, space="PSUM"))

            # constants: [1..8, 1.0] broadcast to all partitions
            cvec = singles.tile([PART, C + 1], f32)
            consts_b = bass.AP(tensor=consts, offset=0, ap=[[0, PART], [1, C + 1]])
            nc.sync.dma_start(out=cvec[:], in_=consts_b)
            # iota copy in PSUM: the mask is_equal then has only one SBUF
            # operand, making it immune to the gpsimd SBUF-port contention.
            iota_ps = psum.tile([PART, C], f32)
            nc.scalar.copy(out=iota_ps[:], in_=cvec[:, 0:C])
            mvec = singles.tile([PART, 1], f32)
            nc.vector.memset(mvec[:], -MSHIFT)

            # stats1[(j,c), (j',(sc|one))] += mask^T @ [logZ | 1]
            # stats2[(j,c), (j',sc)]       += mask^T @ (-U/Z)
            stats_ps = psum.tile([PX_PER_PART * C, PX_PER_PART * EW], f32)
            stats2_ps = psum.tile([PX_PER_PART * C, PX_PER_PART * G], f32)

            # Phase the ACT ops in groups of PH chunks so the activation
            # table only switches twice per group (exp/ln share one set,
            # silu lives in another).
            for g0 in range(0, NCHUNK, PH):
                group = range(g0, min(g0 + PH, NCHUNK))
                a_t, e_t, sy_t = {}, {}, {}
                for ch in group:
                    a = big.tile([PART, FREE], f32, tag="a")
                    nc.sync.dma_start(out=a[:], in_=acts[ch])
                    a_t[ch] = a
                    e = ebuf.tile([PART, FREE], f32, tag="e")
                    nc.scalar.activation(
                        out=e[:], in_=a[:], func=mybir.ActivationFunctionType.Exp
                    )
                    e_t[ch] = e
                for ch in group:
                    # silu(x - m) ~= (x-m)*exp(x-m); exact up to rel err e^(x-m)
                    sy = sybuf.tile([PART, FREE], f32, tag="sy")
                    nc.scalar.activation(
                        out=sy[:],
                        in_=a_t[ch][:],
                        func=mybir.ActivationFunctionType.Silu,
                        bias=mvec[:],
                    )
                    sy_t[ch] = sy
                for ch in group:
                    a, e, sy = a_t[ch], e_t[ch], sy_t[ch]
                    lab = small.tile([PART, PX_PER_PART], f32, tag="lab")
                    nc.sync.dma_start(out=lab[:], in_=labels[ch])

                    z = small.tile([PART, GF], f32, tag="z")
                    nc.vector.tensor_reduce(
                        out=z[:],
                        in_=e[:].rearrange("p (g k) -> p g k", k=K),
                        axis=mybir.AxisListType.X,
                        op=mybir.AluOpType.add,
                    )

                    # mask [128, j=8, c=8] = (label[j] == c+1)
                    mask = small.tile([PART, PX_PER_PART, C], f32, tag="mask")
                    lab_ap = lab[:]
                    lab_b = bass.AP(
                        tensor=lab_ap.tensor,
                        offset=lab_ap.offset,
                        ap=[lab_ap.ap[0], lab_ap.ap[1], [0, C]],
                    )
                    iota_ap = iota_ps[:]
                    iota_b = bass.AP(
                        tensor=iota_ap.tensor,
                        offset=iota_ap.offset,
                        ap=[iota_ap.ap[0], [0, PX_PER_PART], iota_ap.ap[1]],
                    )
                    nc.vector.tensor_tensor(
                        mask[:], lab_b, iota_b, mybir.AluOpType.is_equal
                    )

                    negss = small.tile([PART, GF], f32, tag="negss")
                    if USE_GP_PAIRSUM:
                        # gpsimd halves the silu reduce: sy5 = sy[..,0:5]+sy[..,5:10]
                        sy5 = sy5buf.tile([PART, GF, K // 2], f32, tag="sy5")
                        sy3 = sy[:].rearrange("p (g k) -> p g k", k=K)
                        nc.gpsimd.tensor_add(
                            sy5[:], sy3[:, :, 0 : K // 2], sy3[:, :, K // 2 : K]
                        )
                        nc.vector.tensor_reduce(
                            out=negss[:],
                            in_=sy5[:],
                            axis=mybir.AxisListType.X,
                            op=mybir.AluOpType.add,
                            negate=True,
                        )
                    else:
                        nc.vector.tensor_reduce(
                            out=negss[:],
                            in_=sy[:].rearrange("p (g k) -> p g k", k=K),
                            axis=mybir.AxisListType.X,
                            op=mybir.AluOpType.add,
                            negate=True,
                        )

                    # lz tile [128, j=8, 33]: cols 0..31 = logZ, col 32 = 1.0
                    lz = small.tile([PART, PX_PER_PART, EW], f32, tag="lz")
                    nc.scalar.activation(
                        out=lz[:, :, 0:G],
                        in_=z[:].rearrange("p (j g) -> p j g", g=G),
                        func=mybir.ActivationFunctionType.Ln,
                    )
                    nc.vector.memset(lz[:, :, G : G + 1], 1.0)
                    # 1/Z = exp(-logZ), into PSUM so the multiply below has a
                    # single SBUF operand (gpsimd SBUF-port contention immunity)
                    rz = psum.tile([PART, GF], f32, tag="rz")
                    nc.scalar.activation(
                        out=rz[:],
                        in_=lz[:, :, 0:G],
                        func=mybir.ActivationFunctionType.Exp,
                        scale=-1.0,
                    )

                    meanx = small.tile([PART, GF], f32, tag="meanx")
                    nc.vector.tensor_mul(meanx[:], negss[:], rz[:])

                    nc.tensor.matmul(
                        out=stats_ps[:],
                        lhsT=mask[:].rearrange("p j c -> p (j c)"),
                        rhs=lz[:].rearrange("p j e -> p (j e)"),
                        start=(ch == 0),
                        stop=(ch == NCHUNK - 1),
                        skip_group_check=True,
                    )
                    nc.tensor.matmul(
                        out=stats2_ps[:],
                        lhsT=mask[:].rearrange("p j c -> p (j c)"),
                        rhs=meanx[:],
                        start=(ch == 0),
                        stop=(ch == NCHUNK - 1),
                        skip_group_check=True,
                    )

            stats_sb = singles.tile([PX_PER_PART * C, PX_PER_PART * EW], f32)
            nc.vector.tensor_copy(out=stats_sb[:], in_=stats_ps[:])
            nc.sync.dma_start(out=stats_out, in_=stats_sb[:])
            stats2_sb = singles.tile([PX_PER_PART * C, PX_PER_PART * G], f32)
            nc.vector.tensor_copy(out=stats2_sb[:], in_=stats2_ps[:])
            nc.sync.dma_start(out=stats2_out, in_=stats2_sb[:])

    nc.compile()
    _CACHE["nc"] = nc
    return nc


def _prep_inputs(prototype_activations, target_labels, proto_idx):
    acts = np.asarray(prototype_activations, dtype=np.float32)
    labels = np.asarray(target_labels)
    pidx = np.asarray(proto_idx)

    expected = np.arange(S * C * K, dtype=np.int64).reshape(S, C, K)
    if not np.array_equal(pidx.astype(np.int64), expected):
        # general (slow) fallback: permute proto columns on host
        acts = np.ascontiguousarray(acts[..., pidx.reshape(-1)])

    labels_f = labels.astype(np.float32)
    consts = np.concatenate(
        [np.arange(1, C + 1, dtype=np.float32), np.ones(1, dtype=np.float32)]
    )

    in_maps = []
    for b in range(B):
        in_maps.append(
            {
                "acts": np.ascontiguousarray(acts[b]).reshape(NCHUNK, PART, FREE),
                "labels": np.ascontiguousarray(labels_f[b]).reshape(
                    NCHUNK, PART, PX_PER_PART
                ),
                "consts": consts,
            }
        )
    return in_maps


def _combine(stats_list):
    """stats_list: per-core ([64, 264], [64, 256]) pairs -> final scalar."""
    num = np.zeros((B, S, C), dtype=np.float32)
    cnt = np.zeros((B, C), dtype=np.float32)
    jj = np.arange(PX_PER_PART)
    for b, (st1, st2) in enumerate(stats_list):
        st1 = st1.reshape(PX_PER_PART, C, PX_PER_PART, EW)  # [j, c, j', e]
        d1 = st1[jj, :, jj, :].sum(axis=0)  # [c, e]; e: s*C+c' | count
        st2 = st2.reshape(PX_PER_PART, C, PX_PER_PART, G)
        d2 = st2[jj, :, jj, :].sum(axis=0)  # [c, s*C+c'] of -SS/Z sums
        cntc = d1[:, S * C]
        # ent = logZ - e^m * SS/Z - m  (U = e^m*SS + m*Z)
        ent_cols = (
            d1[:, : S * C].reshape(C, S, C)
            + np.float32(np.exp(MSHIFT)) * d2.reshape(C, S, C)
            - MSHIFT * cntc[:, None, None].astype(np.float32)
        )
        num[b] = ent_cols[np.arange(C), :, np.arange(C)].T  # [s, c]
        cnt[b] = cntc
    num /= np.float32(np.log(K))
    present = cnt > 0
    mean_ent = num / np.maximum(cnt, 1.0)[:, None, :]
    n_entries = np.float32(present.sum() * S)
    total = np.float32((mean_ent * present[:, None, :]).sum(dtype=np.float64))
    if n_entries > 0:
        out = np.float32(total / max(n_entries, np.float32(1.0)))
    else:
        out = np.float32(0.0)
    return out


def kernel(prototype_activations, target_labels, proto_idx, _trace=False, _tmpdir=None):
    nc = _build()
    in_maps = _prep_inputs(prototype_activations, target_labels, proto_idx)
    res = run_bass_kernel_spmd(
        nc, in_maps, list(range(NCORES)), trace=_trace, tmpdir=_tmpdir
    )
    stats_list = [
        (res.results[i]["stats"], res.results[i]["stats2"]) for i in range(NCORES)
    ]
    out = _combine(stats_list)
    if _trace:
        return out, res
    return out


# revision 26
# speedup vs baseline: 1.1473x; 1.1473x over previous
"""Trainium2 Bass kernel for EntropySamplLoss.

Reference semantics (per image b):
  acts [N, P=320] viewed as [N, S=4, C=8, K=10] prototype groups
  ent[n, s, c] = normalized softmax entropy over the K protos of group (s, c)
  num[s, c]   = sum over pixels n with label c of ent[n, s, c]
  cnt[c]      = number of pixels with label c
  loss = mean over present (b, s, c) of num[s, c] / cnt[c]

Device kernel (data-parallel, one image per NeuronCore):
  per chunk of 1024 pixels (tile [128 part, 2560], 8 px per partition):
    E   = exp(x)                       (ACT)
    Z   = group-sum_k E                (DVE grouped tensor_reduce)
    xE  = x * E                        (GPSIMD)
    U   = group-sum_k xE               (DVE grouped tensor_reduce)
    logZ = ln(Z); rZ = exp(-logZ)      (ACT)
    ent_raw = logZ - U*rZ              (DVE)  [= ln(K) * normalized entropy]
    mask[px, (j,c)] = labels==c+1      (DVE is_equal vs broadcast iota)
    stats[ (j,c), (j',sc|ones) ] += mask^T @ [ent|1]   (PE matmul, PSUM accum)
  host: extract diagonal j==j', divide by ln(K), per-class means, final mean.
"""

import sys

if "/opt/trn_rl_repo" not in sys.path:
    sys.path.insert(0, "/opt/trn_rl_repo")

from contextlib import ExitStack

import numpy as np

import concourse.bacc as bacc
import concourse.bass as bass
import concourse.tile as tile
from concourse import mybir
from concourse.bass_utils import run_bass_kernel_spmd
from concourse.tile import add_dep_helper

# Problem shape (hardcoded per spec)
B, N, PP = 8, 65536, 320
S, C, K = 4, 8, 10
NCORES = 8

PX_PER_PART = 8          # pixels per partition ("j" slots)
PART = 128
PX_PER_CHUNK = PART * PX_PER_PART      # 1024
NCHUNK = N // PX_PER_CHUNK             # 64
FREE = PX_PER_PART * PP                # 2560
G = S * C                              # 32 groups per pixel
GF = PX_PER_PART * G                   # 256 group slots per partition
EW = G + 1                             # 33: ent cols + ones col

_CACHE = {}


def _patch_act_tables():
    """Make the combined exp+ln table set the only candidate for Exp/Ln so
    the table-load placement pass doesn't thrash between per-function sets
    (one ACT_TABLE_LOAD total instead of 2 per chunk)."""
    import concourse.hw_specs as hw_specs

    tabs = hw_specs.get_activation_tables("gen3")
    E = mybir.ActivationFunctionType.Exp
    L = mybir.ActivationFunctionType.Ln
    for name, funcs in tabs.items():
        if name != "natural_log_exp_and_others":
            funcs.discard(E)
            funcs.discard(L)


def _build():
    if "nc" in _CACHE:
        return _CACHE["nc"]

    _patch_act_tables()
    f32 = mybir.dt.float32
    nc = bacc.Bacc("TRN2", target_bir_lowering=False, debug=False, num_devices=NCORES)

    acts = nc.dram_tensor("acts", [NCHUNK, PART, FREE], f32, kind="ExternalInput").ap()
    labels = nc.dram_tensor(
        "labels", [NCHUNK, PART, PX_PER_PART], f32, kind="ExternalInput"
    ).ap()
    consts = nc.dram_tensor("consts", [C + 1], f32, kind="ExternalInput")
    stats_out = nc.dram_tensor(
        "stats", [PX_PER_PART * C, PX_PER_PART * EW], f32, kind="ExternalOutput"
    ).ap()
    stats2_out = nc.dram_tensor(
        "stats2", [PX_PER_PART * C, PX_PER_PART * G], f32, kind="ExternalOutput"
    ).ap()

    with tile.TileContext(nc) as tc:
        with ExitStack() as ctx:
            singles = ctx.enter_context(tc.tile_pool(name="singles", bufs=1))
            big = ctx.enter_context(tc.tile_pool(name="big", bufs=3))
            ebuf = ctx.enter_context(tc.tile_pool(name="ebuf", bufs=2))
            xebuf = ctx.enter_context(tc.tile_pool(name="xebuf", bufs=2))
            small = ctx.enter_context(tc.tile_pool(name="small", bufs=3))
            psum = ctx.enter_context(tc.tile_pool(name="psum", bufs=1, space="PSUM"))

            # constants: [1..8, 1.0] broadcast to all partitions
            cvec = singles.tile([PART, C + 1], f32)
            consts_b = bass.AP(tensor=consts, offset=0, ap=[[0, PART], [1, C + 1]])
            nc.sync.dma_start(out=cvec[:], in_=consts_b)
            # iota copy in PSUM: the mask is_equal then has only one SBUF
            # operand, making it immune to the gpsimd SBUF-port contention.
            iota_ps = psum.tile([PART, C], f32)
            nc.scalar.copy(out=iota_ps[:], in_=cvec[:, 0:C])

            # stats1[(j,c), (j',(sc|one))] += mask^T @ [logZ | 1]
            # stats2[(j,c), (j',sc)]       += mask^T @ (-U/Z)
            stats_ps = psum.tile([PX_PER_PART * C, PX_PER_PART * EW], f32)
            stats2_ps = psum.tile([PX_PER_PART * C, PX_PER_PART * G], f32)

            for ch in range(NCHUNK):
                a = big.tile([PART, FREE], f32, tag="a")
                nc.sync.dma_start(out=a[:], in_=acts[ch])
                lab = small.tile([PART, PX_PER_PART], f32, tag="lab")
                nc.sync.dma_start(out=lab[:], in_=labels[ch])

                e = ebuf.tile([PART, FREE], f32, tag="e")
                nc.scalar.activation(
                    out=e[:], in_=a[:], func=mybir.ActivationFunctionType.Exp
                )

                z = small.tile([PART, GF], f32, tag="z")
                nc.vector.tensor_reduce(
                    out=z[:],
                    in_=e[:].rearrange("p (g k) -> p g k", k=K),
                    axis=mybir.AxisListType.X,
                    op=mybir.AluOpType.add,
                )

                # mask [128, j=8, c=8] = (label[j] == c+1)
                mask = small.tile([PART, PX_PER_PART, C], f32, tag="mask")
                lab_ap = lab[:]
                lab_b = bass.AP(
                    tensor=lab_ap.tensor,
                    offset=lab_ap.offset,
                    ap=[lab_ap.ap[0], lab_ap.ap[1], [0, C]],
                )
                iota_ap = iota_ps[:]
                iota_b = bass.AP(
                    tensor=iota_ap.tensor,
                    offset=iota_ap.offset,
                    ap=[iota_ap.ap[0], [0, PX_PER_PART], iota_ap.ap[1]],
                )
                nc.vector.tensor_tensor(
                    mask[:], lab_b, iota_b, mybir.AluOpType.is_equal
                )

                xe = xebuf.tile([PART, FREE], f32, tag="xe")
                nc.gpsimd.tensor_mul(xe[:], a[:], e[:])

                negu = small.tile([PART, GF], f32, tag="negu")
                nc.vector.tensor_reduce(
                    out=negu[:],
                    in_=xe[:].rearrange("p (g k) -> p g k", k=K),
                    axis=mybir.AxisListType.X,
                    op=mybir.AluOpType.add,
                    negate=True,
                )

                # lz tile [128, j=8, 33]: cols 0..31 = logZ, col 32 = 1.0
                lz = small.tile([PART, PX_PER_PART, EW], f32, tag="lz")
                nc.scalar.activation(
                    out=lz[:, :, 0:G],
                    in_=z[:].rearrange("p (j g) -> p j g", g=G),
                    func=mybir.ActivationFunctionType.Ln,
                )
                nc.vector.memset(lz[:, :, G : G + 1], 1.0)
                # 1/Z = exp(-logZ), into PSUM so the meanx multiply below has
                # a single SBUF operand (gpsimd contention immunity).
                rz = psum.tile([PART, GF], f32, tag="rz")
                nc.scalar.activation(
                    out=rz[:],
                    in_=lz[:, :, 0:G],
                    func=mybir.ActivationFunctionType.Exp,
                    scale=-1.0,
                )

                meanx = small.tile([PART, GF], f32, tag="meanx")
                nc.vector.tensor_mul(meanx[:], negu[:], rz[:])

                nc.tensor.matmul(
                    out=stats_ps[:],
                    lhsT=mask[:].rearrange("p j c -> p (j c)"),
                    rhs=lz[:].rearrange("p j e -> p (j e)"),
                    start=(ch == 0),
                    stop=(ch == NCHUNK - 1),
                    skip_group_check=True,
                )
                nc.tensor.matmul(
                    out=stats2_ps[:],
                    lhsT=mask[:].rearrange("p j c -> p (j c)"),
                    rhs=meanx[:],
                    start=(ch == 0),
                    stop=(ch == NCHUNK - 1),
                    skip_group_check=True,
                )

            stats_sb = singles.tile([PX_PER_PART * C, PX_PER_PART * EW], f32)
            nc.vector.tensor_copy(out=stats_sb[:], in_=stats_ps[:])
            nc.sync.dma_start(out=stats_out, in_=stats_sb[:])
            stats2_sb = singles.tile([PX_PER_PART * C, PX_PER_PART * G], f32)
            nc.vector.tensor_copy(out=stats2_sb[:], in_=stats2_ps[:])
            nc.sync.dma_start(out=stats2_out, in_=stats2_sb[:])

    nc.compile()
    _CACHE["nc"] = nc
    return nc


def _prep_inputs(prototype_activations, target_labels, proto_idx):
    acts = np.asarray(prototype_activations, dtype=np.float32)
    labels = np.asarray(target_labels)
    pidx = np.asarray(proto_idx)

    expected = np.arange(S * C * K, dtype=np.int64).reshape(S, C, K)
    if not np.array_equal(pidx.astype(np.int64), expected):
        # general (slow) fallback: permute proto columns on host
        acts = np.ascontiguousarray(acts[..., pidx.reshape(-1)])

    labels_f = labels.astype(np.float32)
    consts = np.concatenate(
        [np.arange(1, C + 1, dtype=np.float32), np.ones(1, dtype=np.float32)]
    )

    in_maps = []
    for b in range(B):
        in_maps.append(
            {
                "acts": np.ascontiguousarray(acts[b]).reshape(NCHUNK, PART, FREE),
                "labels": np.ascontiguousarray(labels_f[b]).reshape(
                    NCHUNK, PART, PX_PER_PART
                ),
                "consts": consts,
            }
        )
    return in_maps


def _combine(stats_list):
    """stats_list: per-core ([64, 264], [64, 256]) pairs -> final scalar."""
    num = np.zeros((B, S, C), dtype=np.float32)
    cnt = np.zeros((B, C), dtype=np.float32)
    jj = np.arange(PX_PER_PART)
    for b, (st1, st2) in enumerate(stats_list):
        st1 = st1.reshape(PX_PER_PART, C, PX_PER_PART, EW)  # [j, c, j', e]
        d1 = st1[jj, :, jj, :].sum(axis=0)  # [c, e]; e: s*C+c' | count
        st2 = st2.reshape(PX_PER_PART, C, PX_PER_PART, G)
        d2 = st2[jj, :, jj, :].sum(axis=0)  # [c, s*C+c'] of -U/Z sums
        ent_cols = d1[:, : S * C].reshape(C, S, C) + d2.reshape(C, S, C)
        num[b] = ent_cols[np.arange(C), :, np.arange(C)].T  # [s, c]
        cnt[b] = d1[:, S * C]
    num /= np.float32(np.log(K))
    present = cnt > 0
    mean_ent = num / np.maximum(cnt, 1.0)[:, None, :]
    n_entries = np.float32(present.sum() * S)
    total = np.float32((mean_ent * present[:, None, :]).sum(dtype=np.float64))
    if n_entries > 0:
        out = np.float32(total / max(n_entries, np.float32(1.0)))
    else:
        out = np.float32(0.0)
    return out


def kernel(prototype_activations, target_labels, proto_idx, _trace=False, _tmpdir=None):
    nc = _build()
    in_maps = _prep_inputs(prototype_activations, target_labels, proto_idx)
    res = run_bass_kernel_spmd(
        nc, in_maps, list(range(NCORES)), trace=_trace, tmpdir=_tmpdir
    )
    stats_list = [
        (res.results[i]["stats"], res.results[i]["stats2"]) for i in range(NCORES)
    ]
    out = _combine(stats_list)
    if _trace:
        return out, res
    return out


# revision 28
# speedup vs baseline: 1.1482x; 1.0008x over previous
"""Trainium2 Bass kernel for EntropySamplLoss.

Reference semantics (per image b):
  acts [N, P=320] viewed as [N, S=4, C=8, K=10] prototype groups
  ent[n, s, c] = normalized softmax entropy over the K protos of group (s, c)
  num[s, c]   = sum over pixels n with label c of ent[n, s, c]
  cnt[c]      = number of pixels with label c
  loss = mean over present (b, s, c) of num[s, c] / cnt[c]

Device kernel (data-parallel, one image per NeuronCore):
  per chunk of 1024 pixels (tile [128 part, 2560], 8 px per partition):
    E    = exp(x)                      (ACT)
    Z    = group-sum_k E               (DVE grouped tensor_reduce, 1-port)
    xE   = x * E                       (GPSIMD, overlaps the DVE reduces)
    -U   = group-sum_k xE, negated     (DVE grouped tensor_reduce, 1-port)
    logZ = ln(Z); rZ = exp(-logZ)      (ACT; rZ lands in PSUM)
    meanx = (-U) * rZ                  (DVE TT; PSUM operand avoids the
                                        gpsimd SBUF-port contention)
    mask[px, (j,c)] = labels==c+1      (DVE is_equal vs PSUM-resident iota)
    stats1[(j,c),(j',sc|1)] += mask^T @ [logZ | 1]   (PE matmul, PSUM accum)
    stats2[(j,c),(j',sc)]   += mask^T @ meanx        (PE matmul, PSUM accum)
  host: diagonal j==j', ent-sums = stats1 + stats2, /ln(K), per-class means,
  final mean over present (image, scale, class) cells.

Engine-contention note: 2-port DVE ops that overlap a GPSIMD tensor op stall
~10x (shared SBUF port). All per-chunk DVE ops here are either 1-port
(tensor_reduce) or have one operand in PSUM, which GPSIMD cannot touch.
The exp/ln activation-table thrash is avoided by patching the table map so
both functions resolve to the combined natural_log_exp_and_others set.
Measured: 405.5 us/core HW exec (DMA roofline ~233 us at 360 GB/s/core;
DVE 93% busy is the bottleneck: 2x grouped reduces at 1x = 5.6 us/chunk).
"""

import sys

if "/opt/trn_rl_repo" not in sys.path:
    sys.path.insert(0, "/opt/trn_rl_repo")

from contextlib import ExitStack

import numpy as np

import concourse.bacc as bacc
import concourse.bass as bass
import concourse.tile as tile
from concourse import mybir
from concourse.bass_utils import run_bass_kernel_spmd
from concourse.tile import add_dep_helper

# Problem shape (hardcoded per spec)
B, N, PP = 8, 65536, 320
S, C, K = 4, 8, 10
NCORES = 8

PX_PER_PART = 8          # pixels per partition ("j" slots)
PART = 128
PX_PER_CHUNK = PART * PX_PER_PART      # 1024
NCHUNK = N // PX_PER_CHUNK             # 64
FREE = PX_PER_PART * PP                # 2560
G = S * C                              # 32 groups per pixel
GF = PX_PER_PART * G                   # 256 group slots per partition
EW = G + 1                             # 33: ent cols + ones col

_CACHE = {}


def _patch_act_tables():
    """Make the combined exp+ln table set the only candidate for Exp/Ln so
    the table-load placement pass doesn't thrash between per-function sets
    (one ACT_TABLE_LOAD total instead of 2 per chunk)."""
    import concourse.hw_specs as hw_specs

    tabs = hw_specs.get_activation_tables("gen3")
    E = mybir.ActivationFunctionType.Exp
    L = mybir.ActivationFunctionType.Ln
    for name, funcs in tabs.items():
        if name != "natural_log_exp_and_others":
            funcs.discard(E)
            funcs.discard(L)


def _build():
    if "nc" in _CACHE:
        return _CACHE["nc"]

    _patch_act_tables()
    f32 = mybir.dt.float32
    nc = bacc.Bacc("TRN2", target_bir_lowering=False, debug=False, num_devices=NCORES)

    acts = nc.dram_tensor("acts", [NCHUNK, PART, FREE], f32, kind="ExternalInput").ap()
    labels = nc.dram_tensor(
        "labels", [NCHUNK, PART, PX_PER_PART], f32, kind="ExternalInput"
    ).ap()
    consts = nc.dram_tensor("consts", [C + 1], f32, kind="ExternalInput")
    stats_out = nc.dram_tensor(
        "stats", [PX_PER_PART * C, PX_PER_PART * EW], f32, kind="ExternalOutput"
    ).ap()
    stats2_out = nc.dram_tensor(
        "stats2", [PX_PER_PART * C, PX_PER_PART * G], f32, kind="ExternalOutput"
    ).ap()

    with tile.TileContext(nc) as tc:
        with ExitStack() as ctx:
            singles = ctx.enter_context(tc.tile_pool(name="singles", bufs=1))
            big = ctx.enter_context(tc.tile_pool(name="big", bufs=4))
            ebuf = ctx.enter_context(tc.tile_pool(name="ebuf", bufs=3))
            xebuf = ctx.enter_context(tc.tile_pool(name="xebuf", bufs=3))
            small = ctx.enter_context(tc.tile_pool(name="small", bufs=4))
            psum = ctx.enter_context(tc.tile_pool(name="psum", bufs=2, space="PSUM"))

            # constants: [1..8, 1.0] broadcast to all partitions
            cvec = singles.tile([PART, C + 1], f32)
            consts_b = bass.AP(tensor=consts, offset=0, ap=[[0, PART], [1, C + 1]])
            nc.sync.dma_start(out=cvec[:], in_=consts_b)
            # iota copy in PSUM: the mask is_equal then has only one SBUF
            # operand, making it immune to the gpsimd SBUF-port contention.
            iota_ps = psum.tile([PART, C], f32)
            nc.scalar.copy(out=iota_ps[:], in_=cvec[:, 0:C])

            # stats1[(j,c), (j',(sc|one))] += mask^T @ [logZ | 1]
            # stats2[(j,c), (j',sc)]       += mask^T @ (-U/Z)
            stats_ps = psum.tile([PX_PER_PART * C, PX_PER_PART * EW], f32)
            stats2_ps = psum.tile([PX_PER_PART * C, PX_PER_PART * G], f32)

            for ch in range(NCHUNK):
                a = big.tile([PART, FREE], f32, tag="a")
                nc.sync.dma_start(out=a[:], in_=acts[ch])
                lab = small.tile([PART, PX_PER_PART], f32, tag="lab")
                nc.sync.dma_start(out=lab[:], in_=labels[ch])

                e = ebuf.tile([PART, FREE], f32, tag="e")
                nc.scalar.activation(
                    out=e[:], in_=a[:], func=mybir.ActivationFunctionType.Exp
                )

                z = small.tile([PART, GF], f32, tag="z")
                nc.vector.tensor_reduce(
                    out=z[:],
                    in_=e[:].rearrange("p (g k) -> p g k", k=K),
                    axis=mybir.AxisListType.X,
                    op=mybir.AluOpType.add,
                )

                # mask [128, j=8, c=8] = (label[j] == c+1)
                mask = small.tile([PART, PX_PER_PART, C], f32, tag="mask")
                lab_ap = lab[:]
                lab_b = bass.AP(
                    tensor=lab_ap.tensor,
                    offset=lab_ap.offset,
                    ap=[lab_ap.ap[0], lab_ap.ap[1], [0, C]],
                )
                iota_ap = iota_ps[:]
                iota_b = bass.AP(
                    tensor=iota_ap.tensor,
                    offset=iota_ap.offset,
                    ap=[iota_ap.ap[0], [0, PX_PER_PART], iota_ap.ap[1]],
                )
                nc.vector.tensor_tensor(
                    mask[:], lab_b, iota_b, mybir.AluOpType.is_equal
                )

                xe = xebuf.tile([PART, FREE], f32, tag="xe")
                nc.gpsimd.tensor_mul(xe[:], a[:], e[:])

                negu = small.tile([PART, GF], f32, tag="negu")
                nc.vector.tensor_reduce(
                    out=negu[:],
                    in_=xe[:].rearrange("p (g k) -> p g k", k=K),
                    axis=mybir.AxisListType.X,
                    op=mybir.AluOpType.add,
                    negate=True,
                )

                # lz tile [128, j=8, 33]: cols 0..31 = logZ, col 32 = 1.0
                lz = small.tile([PART, PX_PER_PART, EW], f32, tag="lz")
                nc.scalar.activation(
                    out=lz[:, :, 0:G],
                    in_=z[:].rearrange("p (j g) -> p j g", g=G),
                    func=mybir.ActivationFunctionType.Ln,
                )
                nc.vector.memset(lz[:, :, G : G + 1], 1.0)
                # 1/Z = exp(-logZ), into PSUM so the meanx multiply below has
                # a single SBUF operand (gpsimd contention immunity).
                rz = psum.tile([PART, GF], f32, tag="rz")
                nc.scalar.activation(
                    out=rz[:],
                    in_=lz[:, :, 0:G],
                    func=mybir.ActivationFunctionType.Exp,
                    scale=-1.0,
                )

                meanx = small.tile([PART, GF], f32, tag="meanx")
                nc.vector.tensor_mul(meanx[:], negu[:], rz[:])

                nc.tensor.matmul(
                    out=stats_ps[:],
                    lhsT=mask[:].rearrange("p j c -> p (j c)"),
                    rhs=lz[:].rearrange("p j e -> p (j e)"),
                    start=(ch == 0),
                    stop=(ch == NCHUNK - 1),
                    skip_group_check=True,
                )
                nc.tensor.matmul(
                    out=stats2_ps[:],
                    lhsT=mask[:].rearrange("p j c -> p (j c)"),
                    rhs=meanx[:],
                    start=(ch == 0),
                    stop=(ch == NCHUNK - 1),
                    skip_group_check=True,
                )

            stats_sb = singles.tile([PX_PER_PART * C, PX_PER_PART * EW], f32)
            nc.vector.tensor_copy(out=stats_sb[:], in_=stats_ps[:])
            nc.sync.dma_start(out=stats_out, in_=stats_sb[:])
            stats2_sb = singles.tile([PX_PER_PART * C, PX_PER_PART * G], f32)
            nc.vector.tensor_copy(out=stats2_sb[:], in_=stats2_ps[:])
            nc.sync.dma_start(out=stats2_out, in_=stats2_sb[:])

    nc.compile()
    _CACHE["nc"] = nc
    return nc


def _prep_inputs(prototype_activations, target_labels, proto_idx):
    acts = np.asarray(prototype_activations, dtype=np.float32)
    labels = np.asarray(target_labels)
    pidx = np.asarray(proto_idx)

    expected = np.arange(S * C * K, dtype=np.int64).reshape(S, C, K)
    if not np.array_equal(pidx.astype(np.int64), expected):
        # general (slow) fallback: permute proto columns on host
        acts = np.ascontiguousarray(acts[..., pidx.reshape(-1)])

    labels_f = labels.astype(np.float32)
    consts = np.concatenate(
        [np.arange(1, C + 1, dtype=np.float32), np.ones(1, dtype=np.float32)]
    )

    in_maps = []
    for b in range(B):
        in_maps.append(
            {
                "acts": np.ascontiguousarray(acts[b]).reshape(NCHUNK, PART, FREE),
                "labels": np.ascontiguousarray(labels_f[b]).reshape(
                    NCHUNK, PART, PX_PER_PART
                ),
                "consts": consts,
            }
        )
    return in_maps


def _combine(stats_list):
    """stats_list: per-core ([64, 264], [64, 256]) pairs -> final scalar."""
    num = np.zeros((B, S, C), dtype=np.float32)
    cnt = np.zeros((B, C), dtype=np.float32)
    jj = np.arange(PX_PER_PART)
    for b, (st1, st2) in enumerate(stats_list):
        st1 = st1.reshape(PX_PER_PART, C, PX_PER_PART, EW)  # [j, c, j', e]
        d1 = st1[jj, :, jj, :].sum(axis=0)  # [c, e]; e: s*C+c' | count
        st2 = st2.reshape(PX_PER_PART, C, PX_PER_PART, G)
        d2 = st2[jj, :, jj, :].sum(axis=0)  # [c, s*C+c'] of -U/Z sums
        ent_cols = d1[:, : S * C].reshape(C, S, C) + d2.reshape(C, S, C)
        num[b] = ent_cols[np.arange(C), :, np.arange(C)].T  # [s, c]
        cnt[b] = d1[:, S * C]
    num /= np.float32(np.log(K))
    present = cnt > 0
    mean_ent = num / np.maximum(cnt, 1.0)[:, None, :]
    n_entries = np.float32(present.sum() * S)
    total = np.float32((mean_ent * present[:, None, :]).sum(dtype=np.float64))
    if n_entries > 0:
        out = np.float32(total / max(n_entries, np.float32(1.0)))
    else:
        out = np.float32(0.0)
    return out


def kernel(prototype_activations, target_labels, proto_idx, _trace=False, _tmpdir=None):
    nc = _build()
    in_maps = _prep_inputs(prototype_activations, target_labels, proto_idx)
    res = run_bass_kernel_spmd(
        nc, in_maps, list(range(NCORES)), trace=_trace, tmpdir=_tmpdir
    )
    stats_list = [
        (res.results[i]["stats"], res.results[i]["stats2"]) for i in range(NCORES)
    ]
    out = _combine(stats_list)
    if _trace:
        return out, res
    return out
